# revision 17
# baseline (speedup 1.0000x reference)
"""Chamfer distance kernel for Trainium2 (8 NeuronCores, Bass/Tile).

Problem: p1, p2 are [B=8, N=4096, D=3] fp32 point clouds. Output is the
scalar  mean_j(min_i P[b,i,j]) + mean_i(min_j P[b,i,j])  where
P[b,i,j] = ||p1[b,i] - p2[b,j]||^2.

Strategy
--------
Data-parallel over B: core b handles batch b.

Points are host-sorted by coordinate 0, so nearest neighbors are close in
rank. Each 128-query block scans the W=64 rank-centered candidates
(window of block i = ranks [128i+32, 128i+96)), for both directions.
Augmented K=24 bf16 operands make each distance block a matmul
(triple-split, fp32-accurate to ~1e-7, see _aug_pair); row minima are
free-axis min reductions on DVE.

The device pipeline (v5) is sized off the v1-v4 traces:
- the DVE tensor_reduce chain is the pacer (1x uop only: 1 elem/cycle/
  lane @0.96GHz, ~150cyc/op overhead, and a strided PSUM read pays for
  the full stride span). W=64 *contiguous* groups ([128, 8, 64] = one
  2KB PSUM bank per group) cut the chain to 8 x ~690ns.
- each group owns its own PSUM bank (8 groups = 8 banks), so the PE
  never write-after-read-stalls on the reducer.
- two row blocks are packed per matmul (K=48: block A's augmented rows
  in the first 24 K-rows, block B's in the next 24; each rhs column is
  zero in the other block's rows). 32 matmuls instead of 64 keep the
  LDWEIGHTS-bound PE (~110ns/MM) ahead of the reduce chain.
- input streams as 4 two-group chunks in consumption order on the sync
  HWDGE ring, one completion semaphore each (a shared counter is a
  race). Each chunk stacks one group on SBUF partitions 0-47 and the
  next on 64-111, so the DMA engages the full 128-partition port set
  (a <=48-partition transfer leaves most SDMA engines idle); compute
  starts after the first ~256KB chunk.
- warmup matmuls at kernel start keep the PE busy while chunk 0 lands.
- the final output DMA's completion is not waited on: it lands during
  the fixed ~7us NRT postamble that follows the block barrier anyway.

Exactness: banding alone misses isolated points. For each row the host
runs an O(1) posterior bound check - every candidate outside the window
has dist^2 >= (coord0 gap to the window edge)^2, so a row whose banded
min is below that gap is *provably* exact. Unproven rows are recomputed
exactly on the host with a radius-bounded rescan (the banded min bounds
the true NN distance, hence its coord0 range).
"""

import sys

import numpy as np

if "/opt/trn_rl_repo" not in sys.path:
    sys.path.insert(0, "/opt/trn_rl_repo")

B = 8
N = 4096
D = 3
W = 64           # band width: window of block i is [128i+32, 128i+96)
WIN_OFF = (128 - W) // 2
NBLK = N // 128  # 32 row blocks per side
GROUP = 8        # blocks per tensor_reduce group (= one 2KB PSUM bank)
NG = 2 * (NBLK // GROUP)  # 8 reduce groups (4 per side)
N_CORES = 8
KAUG = 24        # bf16-split augmented contraction dim (see _aug_pair)
KP = 2 * KAUG    # packed contraction dim: two blocks per matmul
NDUMMY = 8       # N=512 warmup matmuls (~3.4us cold) issued before any wait

_NC_CACHE = {}


def _build_nc():
    """Build the (per-core SPMD) Bass program. Cached per process.

    Raw Bass (no Tile): PE (pack-2 banded matmuls) -> DVE (contiguous
    per-bank min reduces) -> SYNC (chunked DMA in, milestone DMA out),
    with one PSUM bank per reduce group and explicit semaphores.
    """
    if "nc" in _NC_CACHE:
        return _NC_CACHE["nc"]

    import concourse.bass as bass
    import concourse.mybir as mybir

    f32 = mybir.dt.float32
    bf16 = mybir.dt.bfloat16
    nc = bass.Bass()

    GW = GROUP * W   # 512 fp32 = one PSUM bank = one reduce group
    SPAN = 128 * GROUP  # 1024 aug columns per group (512 lhs + 512 rhs)
    HALF = SPAN // 2

    aug_d = nc.dram_tensor("aug", [128, SPAN * NG // 2], bf16, kind="ExternalInput")
    out_d = nc.dram_tensor("mins", [128, NG * GROUP], f32, kind="ExternalOutput")

    with (
        nc.sbuf_tensor("aug_sb", [128, SPAN * NG // 2], bf16) as aug,
        nc.sbuf_tensor("mins_sb", [128, NG * GROUP], f32) as mins,
        nc.psum_tensor("pt_ps", [128, NG * GW], f32) as pt,
        nc.semaphore("ck0") as ck0,
        nc.semaphore("ck1") as ck1,
        nc.semaphore("ck2") as ck2,
        nc.semaphore("ck3") as ck3,
        nc.semaphore("pe_sem") as pe_sem,
        nc.semaphore("dve_sem") as dve_sem,
        nc.semaphore("dma_sem") as dma_sem,
        nc.Block() as block,
    ):
        cks = (ck0, ck1, ck2, ck3)

        def op_ap(gi, col, width):
            # operand AP for group gi: chunk gi//2, K rows at base 64*(gi%2)
            base = 64 * (gi % 2)
            return aug[base : base + KP, SPAN * (gi // 2) + col :][:, 0:width]

        @block.sync
        def _(sync):
            for c in range(NG // 2):
                lo = SPAN * c
                sync.dma_start(
                    aug[:, lo : lo + SPAN], aug_d[:, lo : lo + SPAN]
                ).then_inc(cks[c], 16)
            # stream the output behind the reduces; the last chunk's
            # completion hides under the fixed NRT postamble
            sync.wait_ge(dve_sem, 4)
            sync.dma_start(out_d[:, :32], mins[:, :32]).then_inc(dma_sem, 16)
            sync.wait_ge(dve_sem, 7)
            sync.dma_start(out_d[:, 32:56], mins[:, 32:56]).then_inc(dma_sem, 16)
            sync.wait_ge(dve_sem, 8)
            sync.dma_start(out_d[:, 56:64], mins[:, 56:64]).then_inc(dma_sem, 16)

        @block.tensor
        def _(tensor):
            # HAM warmup on whatever is in SBUF; results overwritten by the
            # real start=True matmuls of group 0 (same PSUM bank, in-order)
            for _ in range(NDUMMY):
                tensor.matmul(
                    pt[:, 0:512], aug[0:KAUG, 0:128], aug[0:KAUG, 0:512],
                    start=True, stop=True,
                )
            for gi in range(NG):
                if gi % 2 == 0:
                    tensor.wait_ge(cks[gi // 2], 16)
                for t in range(GROUP // 2):
                    mm = tensor.matmul(
                        pt[:, gi * GW + 128 * t : gi * GW + 128 * (t + 1)],
                        op_ap(gi, 128 * t, 128),
                        op_ap(gi, HALF + 128 * t, 128),
                        start=True,
                        stop=True,
                    )
                    if t == GROUP // 2 - 1:
                        # MMs complete in pc order; one inc on the last is sound
                        mm.then_inc(pe_sem, 1)

        @block.vector
        def _(vector):
            for gi in range(NG):
                vector.wait_ge(pe_sem, gi + 1)
                vector.tensor_reduce(
                    mins[:, gi * GROUP : (gi + 1) * GROUP],
                    pt[:, gi * GW : (gi + 1) * GW].rearrange(
                        "p (g w) -> p g w", w=W
                    ),
                    axis=mybir.AxisListType.X,
                    op=mybir.AluOpType.min,
                ).then_inc(dve_sem, 1)

    _NC_CACHE["nc"] = nc
    return nc


def _split3(a):
    """Three-level bf16 decomposition: a ~ ah + al + al2 (residual ~2^-27|a|)."""
    import ml_dtypes

    bf = ml_dtypes.bfloat16
    f32 = np.float32
    ah = a.astype(bf).astype(f32)
    r = (a - ah).astype(f32)
    al = r.astype(bf).astype(f32)
    al2 = (r - al).astype(bf).astype(f32)
    return ah, al, al2


def _aug_pair(q, c):
    """bf16-split augmented operands: lhs[:,i] . rhs[:,j] = ||q_i - c_j||^2 / 2.

    All bf16 products are exact in fp32, so accumulating the 6 dominant
    cross terms per coordinate plus triple-split norm rows reproduces the
    fp32 distance to ~1e-7 at bf16 matmul speed.
    """
    f32 = np.float32
    n = len(q)
    lhs_rows, rhs_rows = [], []
    for d in range(D):
        ah, al, al2 = _split3(q[:, d])
        bh, bl, bl2 = _split3(-c[:, d])
        lhs_rows += [ah, ah, al, al, ah, al2]
        rhs_rows += [bh, bl, bh, bl, bl2, bh]
    qd = 0.5 * (q * q).sum(1, dtype=np.float64)
    cd = 0.5 * (c * c).sum(1, dtype=np.float64)
    ones = np.ones(n, f32)
    qh, ql, ql2 = _split3(qd.astype(f32))
    ch, cl, cl2 = _split3(cd.astype(f32))
    lhs_rows += [qh, ql, ql2, ones, ones, ones]
    rhs_rows += [ones, ones, ones, ch, cl, cl2]
    import ml_dtypes

    return (
        np.stack(lhs_rows).astype(ml_dtypes.bfloat16),
        np.stack(rhs_rows).astype(ml_dtypes.bfloat16),
    )


def _prep_batch(x, y):
    """Sort by coord 0 and build the pack-2 chunked operand layout.

    DRAM layout [128, SPAN*NG/2]: chunk c holds group 2c on partition
    rows 0:48 and group 2c+1 on rows 64:112 (full-width DMA). Per group:
    4 lhs pair-tiles [48, 128] then 4 rhs pair-tiles [48, 128]. A pair
    tile covers blocks (2t, 2t+1): lhs column p carries block 2t's
    augmented query p in K-rows 0:24 and block 2t+1's in 24:48; rhs
    columns 0:64 carry block 2t's window (K-rows 0:24, zeros below) and
    columns 64:128 block 2t+1's window (zeros above, K-rows 24:48).
    """
    xs = x[np.argsort(x[:, 0], kind="stable")]
    ys = y[np.argsort(y[:, 0], kind="stable")]

    lhsx, rhsy = _aug_pair(xs, ys)
    lhsy, rhsx = _aug_pair(ys, xs)
    lhs = (lhsx, lhsy)
    rhs = (rhsy, rhsx)

    span = 128 * GROUP  # 1024 columns per group
    half = span // 2
    aug = np.zeros((128, span * NG // 2), dtype=lhsx.dtype)
    for g in range(NG):
        side, gg = divmod(g, NG // 2)
        r0 = 64 * (g % 2)
        for t in range(GROUP // 2):
            ia = 8 * gg + 2 * t
            ib = ia + 1
            lo = span * (g // 2) + 128 * t
            aug[r0 : r0 + KAUG, lo : lo + 128] = lhs[side][
                :, 128 * ia : 128 * (ia + 1)
            ]
            aug[r0 + KAUG : r0 + KP, lo : lo + 128] = lhs[side][
                :, 128 * ib : 128 * (ib + 1)
            ]
            ro = span * (g // 2) + half + 128 * t
            wa = 128 * ia + WIN_OFF
            wb = 128 * ib + WIN_OFF
            aug[r0 : r0 + KAUG, ro : ro + W] = rhs[side][:, wa : wa + W]
            aug[r0 + KAUG : r0 + KP, ro + W : ro + 128] = rhs[side][:, wb : wb + W]
    return xs, ys, {"aug": np.ascontiguousarray(aug)}


def _fix_side(mins, qs, cs):
    """Posterior exactness check + exact host fixup for unproven rows.

    mins: device banded row minima (full P scale) for sorted queries qs
    against sorted candidates cs. Returns exact per-row minima.
    """
    i = np.arange(N) // 128
    lo = np.clip(128 * i + 64 - W // 2, 0, N - W)
    hi = lo + W
    lb = np.full(N, np.inf)
    has_l = lo > 0
    lb[has_l] = np.maximum(0.0, qs[has_l, 0] - cs[lo[has_l] - 1, 0]) ** 2
    has_r = hi < N
    lb[has_r] = np.minimum(
        lb[has_r], np.maximum(0.0, cs[np.minimum(hi[has_r], N - 1), 0] - qs[has_r, 0]) ** 2
    )
    unproven = mins > lb - 1e-5
    if not unproven.any():
        return mins
    # Exact radius-bounded rescan, vectorized in row buckets: the true NN
    # of row r has dist^2 <= mins[r], hence coord0 within +-sqrt(mins[r]).
    rows = np.where(unproven)[0]
    c64 = cs.astype(np.float64)
    c0 = np.ascontiguousarray(c64[:, 0])
    q64 = qs[rows].astype(np.float64)
    rad = np.sqrt(np.maximum(mins[rows], 0.0) + 1e-6)
    jlo = np.searchsorted(c0, q64[:, 0] - rad, "left")
    jhi = np.searchsorted(c0, q64[:, 0] + rad, "right")
    width = jhi - jlo
    out = mins.copy()
    order = np.argsort(width)
    for s in range(0, len(rows), 512):
        sel = order[s : s + 512]
        if width[sel[-1]] == 0:
            continue
        wmax = int(width[sel[-1]])
        idx = jlo[sel, None] + np.arange(wmax)[None, :]
        valid = idx < jhi[sel, None]
        idx = np.minimum(idx, N - 1)
        dd = c64[idx] - q64[sel, None, :]
        d2 = (dd * dd).sum(-1)
        d2[~valid] = np.inf
        out[rows[sel]] = np.minimum(out[rows[sel]], d2.min(1))
    return out


def _postprocess(results, meta):
    """Combine per-core device outputs into the final scalar."""
    total = 0.0
    half = NG // 2
    for b in range(B):
        xs, ys = meta[b]
        m = results[b]["mins"]  # [128, NG*GROUP]; [p, s*32+i] = min for rank 128i+p
        mx = 2.0 * np.ascontiguousarray(m[:, : half * GROUP].T).reshape(N)
        my = 2.0 * np.ascontiguousarray(m[:, half * GROUP :].T).reshape(N)
        mx = _fix_side(mx, xs, ys)
        my = _fix_side(my, ys, xs)
        total += mx.mean(dtype=np.float64) + my.mean(dtype=np.float64)
    return np.array(total / B, dtype=np.float32)


def _run(inputs, trace=False):
    p1 = np.ascontiguousarray(np.asarray(inputs["p1"], dtype=np.float32))
    p2 = np.ascontiguousarray(np.asarray(inputs["p2"], dtype=np.float32))
    assert p1.shape == (B, N, D) and p2.shape == (B, N, D)

    in_maps = []
    meta = []
    for b in range(B):
        xs, ys, im = _prep_batch(p1[b], p2[b])
        in_maps.append(im)
        meta.append((xs, ys))

    from concourse.bass_utils import run_bass_kernel_spmd

    nc = _build_nc()
    kw = {}
    if trace:
        kw = dict(trace=True, trace_cores=list(range(N_CORES)))
    res = run_bass_kernel_spmd(nc, in_maps, list(range(N_CORES)), **kw)
    return _postprocess(res.results, meta), res


def kernel(**inputs):
    out, _ = _run(inputs, trace=False)
    return out


def kernel_traced(**inputs):
    """Same as kernel() but also returns BassKernelResults with NTFF timing."""
    return _run(inputs, trace=True)


# revision 18
# speedup vs baseline: 1.0974x; 1.0974x over previous
"""Chamfer distance kernel for Trainium2 (8 NeuronCores, Bass/Tile).

Problem: p1, p2 are [B=8, N=4096, D=3] fp32 point clouds. Output is the
scalar  mean_j(min_i P[b,i,j]) + mean_i(min_j P[b,i,j])  where
P[b,i,j] = ||p1[b,i] - p2[b,j]||^2.

Strategy
--------
Data-parallel over B: core b handles batch b.

Points are host-sorted by coordinate 0, so nearest neighbors are close in
rank. Each 128-query block scans the W=64 rank-centered candidates
(window of block i = ranks [128i+32, 128i+96)), for both directions.
Augmented K=15 bf16 operands make each distance block a matmul
(split-bf16, ~1e-4-accurate distances - the 2e-2 gate and the exact host
fixup leave large margin; see _aug_pair); row minima are free-axis min
reductions on DVE.

The device pipeline (v5) is sized off the v1-v4 traces:
- the DVE tensor_reduce chain is the pacer (1x uop only: 1 elem/cycle/
  lane @0.96GHz, ~150cyc/op overhead, and a strided PSUM read pays for
  the full stride span). W=64 *contiguous* groups ([128, 8, 64] = one
  2KB PSUM bank per group) cut the chain to 8 x ~690ns.
- each group owns its own PSUM bank (8 groups = 8 banks), so the PE
  never write-after-read-stalls on the reducer.
- two row blocks are packed per matmul (K=48: block A's augmented rows
  in the first 24 K-rows, block B's in the next 24; each rhs column is
  zero in the other block's rows). 32 matmuls instead of 64 keep the
  LDWEIGHTS-bound PE (~110ns/MM) ahead of the reduce chain.
- input streams as 2 four-group chunks (x side, then y side) on the
  sync HWDGE ring, one completion semaphore each (a shared counter is a
  race). Each chunk stacks four groups on 32-partition-row slots (K=30
  packed operands at base partitions 0/32/64/96 - the fourth slot needs
  an explicit tile_position), so the DMA engages the full 128-partition
  port set and the whole input is only ~512KB per core: all 8 cores
  stream simultaneously, so bytes = HBM contention = arrival variance
  on the slowest core.
- warmup matmuls at kernel start keep the PE busy while chunk 0 lands.
- the final output DMA's completion is not waited on: it lands during
  the fixed ~7us NRT postamble that follows the block barrier anyway.

Exactness: banding alone misses isolated points. For each row the host
runs an O(1) posterior bound check - every candidate outside the window
has dist^2 >= (coord0 gap to the window edge)^2, so a row whose banded
min is below that gap is *provably* exact. Unproven rows are recomputed
exactly on the host with a radius-bounded rescan (the banded min bounds
the true NN distance, hence its coord0 range).
"""

import sys

import numpy as np

if "/opt/trn_rl_repo" not in sys.path:
    sys.path.insert(0, "/opt/trn_rl_repo")

B = 8
N = 4096
D = 3
W = 64           # band width: window of block i is [128i+32, 128i+96)
WIN_OFF = (128 - W) // 2
NBLK = N // 128  # 32 row blocks per side
GROUP = 8        # blocks per tensor_reduce group (= one 2KB PSUM bank)
NG = 2 * (NBLK // GROUP)  # 8 reduce groups (4 per side)
N_CORES = 8
KAUG = 15        # bf16-split augmented contraction dim (see _aug_pair)
KP = 2 * KAUG    # packed contraction dim: two blocks per matmul
NDUMMY = 8       # N=512 warmup matmuls (~3.4us cold) issued before any wait

_NC_CACHE = {}


def _build_nc():
    """Build the (per-core SPMD) Bass program. Cached per process.

    Raw Bass (no Tile): PE (pack-2 banded matmuls) -> DVE (contiguous
    per-bank min reduces) -> SYNC (chunked DMA in, milestone DMA out),
    with one PSUM bank per reduce group and explicit semaphores.
    """
    if "nc" in _NC_CACHE:
        return _NC_CACHE["nc"]

    import concourse.bass as bass
    import concourse.mybir as mybir

    f32 = mybir.dt.float32
    bf16 = mybir.dt.bfloat16
    nc = bass.Bass()

    GW = GROUP * W   # 512 fp32 = one PSUM bank = one reduce group
    SPAN = 128 * GROUP  # 1024 aug columns per group (512 lhs + 512 rhs)
    HALF = SPAN // 2

    aug_d = nc.dram_tensor("aug", [128, SPAN * NG // 4], bf16, kind="ExternalInput")
    out_d = nc.dram_tensor("mins", [128, NG * GROUP], f32, kind="ExternalOutput")

    with (
        nc.sbuf_tensor("aug_sb", [128, SPAN * NG // 4], bf16) as aug,
        nc.sbuf_tensor("mins_sb", [128, NG * GROUP], f32) as mins,
        nc.psum_tensor("pt_ps", [128, NG * GW], f32) as pt,
        nc.semaphore("ck0") as ck0,
        nc.semaphore("ck1") as ck1,
        nc.semaphore("pe_sem") as pe_sem,
        nc.semaphore("dve_sem") as dve_sem,
        nc.semaphore("dma_sem") as dma_sem,
        nc.Block() as block,
    ):
        cks = (ck0, ck1)

        def op_ap(gi, col, width):
            # operand AP for group gi: chunk gi//4, K rows at base 32*(gi%4)
            base = 32 * (gi % 4)
            return aug[base : base + KP, SPAN * (gi // 4) + col :][:, 0:width]

        @block.sync
        def _(sync):
            for c in range(NG // 4):
                lo = SPAN * c
                sync.dma_start(
                    aug[:, lo : lo + SPAN], aug_d[:, lo : lo + SPAN]
                ).then_inc(cks[c], 16)
            # stream the output behind the reduces; the last chunk's
            # completion hides under the fixed NRT postamble
            sync.wait_ge(dve_sem, 4)
            sync.dma_start(out_d[:, :32], mins[:, :32]).then_inc(dma_sem, 16)
            sync.wait_ge(dve_sem, 7)
            sync.dma_start(out_d[:, 32:56], mins[:, 32:56]).then_inc(dma_sem, 16)
            sync.wait_ge(dve_sem, 8)
            sync.dma_start(out_d[:, 56:64], mins[:, 56:64]).then_inc(dma_sem, 16)

        @block.tensor
        def _(tensor):
            # HAM warmup on whatever is in SBUF; results overwritten by the
            # real start=True matmuls of group 0 (same PSUM bank, in-order)
            for _ in range(NDUMMY):
                tensor.matmul(
                    pt[:, 0:512], aug[0:KAUG, 0:128], aug[0:KAUG, 0:512],
                    start=True, stop=True,
                )
            for gi in range(NG):
                if gi % 4 == 0:
                    tensor.wait_ge(cks[gi // 4], 16)
                tp = (96, 0) if gi % 4 == 3 else None
                for t in range(GROUP // 2):
                    mm = tensor.matmul(
                        pt[:, gi * GW + 128 * t : gi * GW + 128 * (t + 1)],
                        op_ap(gi, 128 * t, 128),
                        op_ap(gi, HALF + 128 * t, 128),
                        start=True,
                        stop=True,
                        tile_position=tp,
                    )
                    if t == GROUP // 2 - 1:
                        # MMs complete in pc order; one inc on the last is sound
                        mm.then_inc(pe_sem, 1)

        @block.vector
        def _(vector):
            for gi in range(NG):
                vector.wait_ge(pe_sem, gi + 1)
                vector.tensor_reduce(
                    mins[:, gi * GROUP : (gi + 1) * GROUP],
                    pt[:, gi * GW : (gi + 1) * GW].rearrange(
                        "p (g w) -> p g w", w=W
                    ),
                    axis=mybir.AxisListType.X,
                    op=mybir.AluOpType.min,
                ).then_inc(dve_sem, 1)

    _NC_CACHE["nc"] = nc
    return nc


def _split3(a):
    """Three-level bf16 decomposition: a ~ ah + al + al2 (residual ~2^-27|a|)."""
    import ml_dtypes

    bf = ml_dtypes.bfloat16
    f32 = np.float32
    ah = a.astype(bf).astype(f32)
    r = (a - ah).astype(f32)
    al = r.astype(bf).astype(f32)
    al2 = (r - al).astype(bf).astype(f32)
    return ah, al, al2


def _aug_pair(q, c):
    """bf16-split augmented operands: lhs[:,i] . rhs[:,j] = ||q_i - c_j||^2 / 2.

    All bf16 products are exact in fp32; the 3 dominant cross terms per
    coordinate plus triple-split norm rows give ~1e-4-accurate distances
    at bf16 matmul speed with only K=15 rows (KP=30 packed <= 32, so four
    groups share the 128 partition rows).
    """
    f32 = np.float32
    n = len(q)
    lhs_rows, rhs_rows = [], []
    for d in range(D):
        ah, al, _ = _split3(q[:, d])
        bh, bl, _ = _split3(-c[:, d])
        lhs_rows += [ah, ah, al]
        rhs_rows += [bh, bl, bh]
    qd = 0.5 * (q * q).sum(1, dtype=np.float64)
    cd = 0.5 * (c * c).sum(1, dtype=np.float64)
    ones = np.ones(n, f32)
    qh, ql, ql2 = _split3(qd.astype(f32))
    ch, cl, cl2 = _split3(cd.astype(f32))
    lhs_rows += [qh, ql, ql2, ones, ones, ones]
    rhs_rows += [ones, ones, ones, ch, cl, cl2]
    import ml_dtypes

    return (
        np.stack(lhs_rows).astype(ml_dtypes.bfloat16),
        np.stack(rhs_rows).astype(ml_dtypes.bfloat16),
    )


def _prep_batch(x, y):
    """Sort by coord 0 and build the pack-2 chunked operand layout.

    DRAM layout [128, SPAN*NG/4]: chunk c holds groups 4c..4c+3 on
    32-partition-row slots (full-width DMA). Per group: 4 lhs pair-tiles
    [30, 128] then 4 rhs pair-tiles [30, 128]. A pair tile covers blocks
    (2t, 2t+1): lhs column p carries block 2t's augmented query p in
    K-rows 0:15 and block 2t+1's in 15:30; rhs columns 0:64 carry block
    2t's window (K-rows 0:15, zeros below) and columns 64:128 block
    2t+1's window (zeros above, K-rows 15:30).
    """
    xs = x[np.argsort(x[:, 0], kind="stable")]
    ys = y[np.argsort(y[:, 0], kind="stable")]

    lhsx, rhsy = _aug_pair(xs, ys)
    lhsy, rhsx = _aug_pair(ys, xs)
    lhs = (lhsx, lhsy)
    rhs = (rhsy, rhsx)

    span = 128 * GROUP  # 1024 columns per group
    half = span // 2
    aug = np.zeros((128, span * NG // 4), dtype=lhsx.dtype)
    for g in range(NG):
        side, gg = divmod(g, NG // 2)
        r0 = 32 * (g % 4)
        for t in range(GROUP // 2):
            ia = 8 * gg + 2 * t
            ib = ia + 1
            lo = span * (g // 4) + 128 * t
            aug[r0 : r0 + KAUG, lo : lo + 128] = lhs[side][
                :, 128 * ia : 128 * (ia + 1)
            ]
            aug[r0 + KAUG : r0 + KP, lo : lo + 128] = lhs[side][
                :, 128 * ib : 128 * (ib + 1)
            ]
            ro = span * (g // 4) + half + 128 * t
            wa = 128 * ia + WIN_OFF
            wb = 128 * ib + WIN_OFF
            aug[r0 : r0 + KAUG, ro : ro + W] = rhs[side][:, wa : wa + W]
            aug[r0 + KAUG : r0 + KP, ro + W : ro + 128] = rhs[side][:, wb : wb + W]
    return xs, ys, {"aug": np.ascontiguousarray(aug)}


def _fix_side(mins, qs, cs):
    """Posterior exactness check + exact host fixup for unproven rows.

    mins: device banded row minima (full P scale) for sorted queries qs
    against sorted candidates cs. Returns exact per-row minima.
    """
    i = np.arange(N) // 128
    lo = np.clip(128 * i + 64 - W // 2, 0, N - W)
    hi = lo + W
    lb = np.full(N, np.inf)
    has_l = lo > 0
    lb[has_l] = np.maximum(0.0, qs[has_l, 0] - cs[lo[has_l] - 1, 0]) ** 2
    has_r = hi < N
    lb[has_r] = np.minimum(
        lb[has_r], np.maximum(0.0, cs[np.minimum(hi[has_r], N - 1), 0] - qs[has_r, 0]) ** 2
    )
    unproven = mins > lb - 3e-4  # margin covers the K=15 distance error
    if not unproven.any():
        return mins
    # Exact radius-bounded rescan, vectorized in row buckets: the true NN
    # of row r has dist^2 <= mins[r], hence coord0 within +-sqrt(mins[r]).
    rows = np.where(unproven)[0]
    c64 = cs.astype(np.float64)
    c0 = np.ascontiguousarray(c64[:, 0])
    q64 = qs[rows].astype(np.float64)
    rad = np.sqrt(np.maximum(mins[rows], 0.0) + 1e-6)
    jlo = np.searchsorted(c0, q64[:, 0] - rad, "left")
    jhi = np.searchsorted(c0, q64[:, 0] + rad, "right")
    width = jhi - jlo
    out = mins.copy()
    order = np.argsort(width)
    for s in range(0, len(rows), 512):
        sel = order[s : s + 512]
        if width[sel[-1]] == 0:
            continue
        wmax = int(width[sel[-1]])
        idx = jlo[sel, None] + np.arange(wmax)[None, :]
        valid = idx < jhi[sel, None]
        idx = np.minimum(idx, N - 1)
        dd = c64[idx] - q64[sel, None, :]
        d2 = (dd * dd).sum(-1)
        d2[~valid] = np.inf
        out[rows[sel]] = np.minimum(out[rows[sel]], d2.min(1))
    return out


def _postprocess(results, meta):
    """Combine per-core device outputs into the final scalar."""
    total = 0.0
    half = NG // 2
    for b in range(B):
        xs, ys = meta[b]
        m = results[b]["mins"]  # [128, NG*GROUP]; [p, s*32+i] = min for rank 128i+p
        mx = 2.0 * np.ascontiguousarray(m[:, : half * GROUP].T).reshape(N)
        my = 2.0 * np.ascontiguousarray(m[:, half * GROUP :].T).reshape(N)
        mx = _fix_side(mx, xs, ys)
        my = _fix_side(my, ys, xs)
        total += mx.mean(dtype=np.float64) + my.mean(dtype=np.float64)
    return np.array(total / B, dtype=np.float32)


def _run(inputs, trace=False):
    p1 = np.ascontiguousarray(np.asarray(inputs["p1"], dtype=np.float32))
    p2 = np.ascontiguousarray(np.asarray(inputs["p2"], dtype=np.float32))
    assert p1.shape == (B, N, D) and p2.shape == (B, N, D)

    in_maps = []
    meta = []
    for b in range(B):
        xs, ys, im = _prep_batch(p1[b], p2[b])
        in_maps.append(im)
        meta.append((xs, ys))

    from concourse.bass_utils import run_bass_kernel_spmd

    nc = _build_nc()
    kw = {}
    if trace:
        kw = dict(trace=True, trace_cores=list(range(N_CORES)))
    res = run_bass_kernel_spmd(nc, in_maps, list(range(N_CORES)), **kw)
    return _postprocess(res.results, meta), res


def kernel(**inputs):
    out, _ = _run(inputs, trace=False)
    return out


def kernel_traced(**inputs):
    """Same as kernel() but also returns BassKernelResults with NTFF timing."""
    return _run(inputs, trace=True)
